# revision 1
# baseline (speedup 1.0000x reference)
"""CLIP-MLP contrastive loss kernel for 8 Trainium2 NeuronCores.

Problem (see reference): B=4096, D_IN=512, D_HID=1024, D_OUT=512, N_CLS=32000.
  h   = relu(img @ W1 + b1)
  u   = h @ W2 + b2                       (called `mlp` in the reference)
  z   = u @ txt                           [B, N_CLS]
  After the reference's normalizations, sim == z / ||z||_row exactly
  (exp(logit_scale) and ||u||_row cancel), so with v = z / (t*||z||):
     loss = mean_b( LSE(v_b) - v_b[tgt_b] ),   acc = sum_b(argmax z_b == tgt_b)
  Because ||v_b||_2 = 1/t (tiny entries), LSE is recovered on the host from
  row statistics only:  sum_c exp(v) = N + (sum_c z)*s + 0.5/t^2 + O(1e-9),
  s = 1/(t*sqrt(sum z^2)).  The device therefore only computes, per row:
     max(z), sum(z^2), z[tgt], sum(z)
  streamed out of PSUM while the big matmul runs - z is never materialized.

Sharding: data-parallel over the batch; 512 rows per core; weights and txt
replicated (txt pre-cast to bf16 on the host). All matmuls bf16 with f32
PSUM accumulation.
"""

import numpy as np
import ml_dtypes

import concourse.bass as bass
import concourse.tile as tile
from concourse import bacc, mybir
from concourse.bass_utils import run_bass_kernel_spmd

BF16 = mybir.dt.bfloat16
F32 = mybir.dt.float32
AF = mybir.ActivationFunctionType
ALU = mybir.AluOpType

N_CORES = 8
B, D_IN, D_HID, D_OUT, N_CLS = 4096, 512, 1024, 512, 32000
B_LOC = B // N_CORES          # 512 rows per core
M_TILES = B_LOC // 128        # 4
KI = D_IN // 128              # 4  k-chunks for layer 1
KH = D_HID // 128             # 8  k-chunks for layer 2
KO = D_OUT // 128             # 4  k-chunks for the big matmul
GROUP = 2048                  # columns of txt processed per PSUM tile
N_GROUPS = (N_CLS + GROUP - 1) // GROUP   # 16 (last group is 1280)

_CACHED_NC = None


def _build_nc():
    nc = bacc.Bacc(None, target_bir_lowering=False, debug=False)

    xt = nc.dram_tensor("xt", [D_IN, B_LOC], BF16, kind="ExternalInput")
    w1 = nc.dram_tensor("w1", [D_IN, D_HID], BF16, kind="ExternalInput")
    b1 = nc.dram_tensor("b1", [D_HID], F32, kind="ExternalInput")
    w2 = nc.dram_tensor("w2", [D_HID, D_OUT], BF16, kind="ExternalInput")
    b2 = nc.dram_tensor("b2", [D_OUT], F32, kind="ExternalInput")
    b2r = nc.dram_tensor("b2r", [128, D_OUT], F32, kind="ExternalInput")
    txt = nc.dram_tensor("txt", [D_OUT, N_CLS], BF16, kind="ExternalInput")
    tgr = nc.dram_tensor("tgr", [B_LOC, D_OUT], BF16, kind="ExternalInput")
    t1r = nc.dram_tensor("t1r", [128, D_OUT], BF16, kind="ExternalInput")

    o_max = nc.dram_tensor("o_max", [B_LOC], F32, kind="ExternalOutput")
    o_ss = nc.dram_tensor("o_ss", [B_LOC], F32, kind="ExternalOutput")
    o_tgt = nc.dram_tensor("o_tgt", [B_LOC], F32, kind="ExternalOutput")
    o_rs = nc.dram_tensor("o_rs", [B_LOC], F32, kind="ExternalOutput")

    with tile.TileContext(nc) as tc:
        with (
            tc.tile_pool(name="weights", bufs=1) as wpool,
            tc.tile_pool(name="acts", bufs=1) as apool,
            tc.tile_pool(name="txtp", bufs=3) as txtpool,
            tc.tile_pool(name="scratch", bufs=2) as scr,
            tc.tile_pool(name="psum", bufs=2, space="PSUM") as ps,
        ):
            # ---- load inputs ----
            xt_sb = wpool.tile([128, KI, B_LOC], BF16, tag="xt")
            w1_sb = wpool.tile([128, KI, D_HID], BF16, tag="w1")
            b1_sb = wpool.tile([128, KH], F32, tag="b1")
            w2_sb = wpool.tile([128, KH, D_OUT], BF16, tag="w2")
            b2_sb = wpool.tile([128, KO], F32, tag="b2")
            b2r_sb = wpool.tile([128, D_OUT], F32, tag="b2r")
            tgr_sb = wpool.tile([128, M_TILES, D_OUT], BF16, tag="tgr")
            t1r_sb = wpool.tile([128, D_OUT], BF16, tag="t1r")

            nc.sync.dma_start(out=xt_sb, in_=xt[:].rearrange("(k p) b -> p k b", p=128))
            nc.sync.dma_start(out=w1_sb, in_=w1[:].rearrange("(k p) m -> p k m", p=128))
            nc.sync.dma_start(out=b1_sb, in_=b1[:].rearrange("(k p) -> p k", p=128))
            nc.sync.dma_start(out=w2_sb, in_=w2[:].rearrange("(k p) n -> p k n", p=128))
            nc.sync.dma_start(out=b2_sb, in_=b2[:].rearrange("(k p) -> p k", p=128))
            nc.sync.dma_start(out=b2r_sb, in_=b2r[:])
            nc.sync.dma_start(out=tgr_sb, in_=tgr[:].rearrange("(m p) d -> p m d", p=128))
            nc.sync.dma_start(out=t1r_sb, in_=t1r[:])

            # ---- layer 1: hT = relu(W1.T @ X + b1)   [D_HID, B_LOC] ----
            h_sb = apool.tile([128, KH, B_LOC], BF16, tag="h")
            for m in range(KH):
                hp = ps.tile([128, GROUP], F32, tag="z", name=f"hp{m}")
                for k in range(KI):
                    nc.tensor.matmul(
                        hp[:, 0:B_LOC],
                        w1_sb[:, k, m * 128 : (m + 1) * 128],
                        xt_sb[:, k, :],
                        start=(k == 0),
                        stop=(k == KI - 1),
                    )
                nc.scalar.activation(
                    out=h_sb[:, m, :], in_=hp[:, 0:B_LOC],
                    func=AF.Relu, bias=b1_sb[:, m : m + 1], scale=1.0,
                )

            # ---- layer 2a: uT = W2.T @ hT + b2   [D_OUT, B_LOC] (lhsT of z) ----
            ut_sb = apool.tile([128, KO, B_LOC], BF16, tag="ut")
            for m in range(KO):
                up = ps.tile([128, GROUP], F32, tag="z", name=f"up{m}")
                for k in range(KH):
                    nc.tensor.matmul(
                        up[:, 0:B_LOC],
                        w2_sb[:, k, m * 128 : (m + 1) * 128],
                        h_sb[:, k, :],
                        start=(k == 0),
                        stop=(k == KH - 1),
                    )
                nc.scalar.activation(
                    out=ut_sb[:, m, :], in_=up[:, 0:B_LOC],
                    func=AF.Identity, bias=b2_sb[:, m : m + 1], scale=1.0,
                )

            # ---- layer 2b: u_row = hT.T @ W2 + b2   [B_LOC, D_OUT] (row layout,
            #      for the per-row dot products against gathered txt columns) ----
            urow_sb = apool.tile([128, M_TILES, D_OUT], BF16, tag="urow")
            for m in range(M_TILES):
                rp = ps.tile([128, GROUP], F32, tag="z", name=f"rp{m}")
                for k in range(KH):
                    nc.tensor.matmul(
                        rp[:, 0:D_OUT],
                        h_sb[:, k, m * 128 : (m + 1) * 128],
                        w2_sb[:, k, :],
                        start=(k == 0),
                        stop=(k == KH - 1),
                    )
                nc.vector.tensor_tensor(
                    out=urow_sb[:, m, :], in0=rp[:, 0:D_OUT], in1=b2r_sb[:],
                    op=ALU.add,
                )

            # ---- per-row dots: z[b, tgt_b] and sum_c z[b, c] ----
            tgt_sl = apool.tile([128, M_TILES], F32, tag="tgt_sl")
            rs_sl = apool.tile([128, M_TILES], F32, tag="rs_sl")
            for m in range(M_TILES):
                prod = scr.tile([128, D_OUT], F32, tag="prod", name=f"pr{m}")
                nc.vector.scalar_tensor_tensor(
                    out=prod, in0=urow_sb[:, m, :], scalar=1.0,
                    in1=tgr_sb[:, m, :], op0=ALU.mult, op1=ALU.mult,
                    accum_out=tgt_sl[:, m : m + 1],
                )
                prod2 = scr.tile([128, D_OUT], F32, tag="prod", name=f"pr2{m}")
                nc.vector.scalar_tensor_tensor(
                    out=prod2, in0=urow_sb[:, m, :], scalar=1.0,
                    in1=t1r_sb[:], op0=ALU.mult, op1=ALU.mult,
                    accum_out=rs_sl[:, m : m + 1],
                )

            # ---- main loop: z = uT.T @ txt, streamed; per-row max + sumsq ----
            max_sl = apool.tile([128, M_TILES, N_GROUPS], F32, tag="max_sl")
            ss_sl = apool.tile([128, M_TILES, N_GROUPS], F32, tag="ss_sl")
            for g in range(N_GROUPS):
                g0 = g * GROUP
                gw = min(GROUP, N_CLS - g0)
                tx = txtpool.tile([128, KO, GROUP], BF16, tag="tx", name=f"tx{g}")
                nc.sync.dma_start(
                    out=tx[:, :, 0:gw],
                    in_=txt[:, g0 : g0 + gw].rearrange("(k p) c -> p k c", p=128),
                )
                for m in range(M_TILES):
                    zp = ps.tile([128, GROUP], F32, tag="z", name=f"zp{g}_{m}")
                    for k in range(KO):
                        for n0 in range(0, gw, 512):
                            nw = min(512, gw - n0)
                            nc.tensor.matmul(
                                zp[:, n0 : n0 + nw],
                                ut_sb[:, k, m * 128 : (m + 1) * 128],
                                tx[:, k, n0 : n0 + nw],
                                start=(k == 0),
                                stop=(k == KO - 1),
                            )
                    nc.vector.tensor_reduce(
                        out=max_sl[:, m, g : g + 1], in_=zp[:, 0:gw],
                        axis=mybir.AxisListType.X, op=ALU.max,
                    )
                    sq = scr.tile([128, GROUP], BF16, tag="sq", name=f"sq{g}_{m}")
                    nc.scalar.activation(
                        out=sq[:, 0:gw], in_=zp[:, 0:gw], func=AF.Square,
                        accum_out=ss_sl[:, m, g : g + 1],
                    )

            # ---- finals + outputs ----
            fin_max = apool.tile([128, M_TILES], F32, tag="fin_max")
            fin_ss = apool.tile([128, M_TILES], F32, tag="fin_ss")
            for m in range(M_TILES):
                nc.vector.tensor_reduce(
                    out=fin_max[:, m : m + 1], in_=max_sl[:, m, :],
                    axis=mybir.AxisListType.X, op=ALU.max,
                )
                nc.vector.tensor_reduce(
                    out=fin_ss[:, m : m + 1], in_=ss_sl[:, m, :],
                    axis=mybir.AxisListType.X, op=ALU.add,
                )
            nc.sync.dma_start(out=o_max[:].rearrange("(m p) -> p m", p=128), in_=fin_max)
            nc.sync.dma_start(out=o_ss[:].rearrange("(m p) -> p m", p=128), in_=fin_ss)
            nc.sync.dma_start(out=o_tgt[:].rearrange("(m p) -> p m", p=128), in_=tgt_sl)
            nc.sync.dma_start(out=o_rs[:].rearrange("(m p) -> p m", p=128), in_=rs_sl)

    nc.compile()
    return nc


def get_nc():
    global _CACHED_NC
    if _CACHED_NC is None:
        _CACHED_NC = _build_nc()
    return _CACHED_NC


def make_in_maps(img_features, txt_features, target_ind, W1, b1, W2, b2):
    bf16 = ml_dtypes.bfloat16
    txt_bf = np.ascontiguousarray(txt_features.astype(bf16))
    w1_bf = np.ascontiguousarray(W1.astype(bf16))
    w2_bf = np.ascontiguousarray(W2.astype(bf16))
    b1_f = np.ascontiguousarray(b1.astype(np.float32))
    b2_f = np.ascontiguousarray(b2.astype(np.float32))
    b2r = np.ascontiguousarray(np.broadcast_to(b2_f, (128, D_OUT)))
    t1 = txt_features.astype(np.float64).sum(axis=1).astype(np.float32).astype(bf16)
    t1r = np.ascontiguousarray(np.broadcast_to(t1, (128, D_OUT)))

    in_maps = []
    for c in range(N_CORES):
        rows = slice(c * B_LOC, (c + 1) * B_LOC)
        xt_c = np.ascontiguousarray(img_features[rows].T.astype(bf16))
        tg_c = target_ind[rows]
        # rows of tgr are the gathered txt columns, in the SAME bf16 values
        # the device multiplies with, so the argmax comparison is consistent
        tgr_c = np.ascontiguousarray(txt_bf[:, tg_c].T)
        in_maps.append({
            "xt": xt_c, "w1": w1_bf, "b1": b1_f, "w2": w2_bf, "b2": b2_f,
            "b2r": b2r, "txt": txt_bf, "tgr": tgr_c, "t1r": t1r,
        })
    return in_maps


def postprocess(results, target_count, t):
    """Combine per-core row statistics into (loss, acc) on the host."""
    maxz = np.concatenate([r["o_max"] for r in results]).astype(np.float64)
    ss = np.concatenate([r["o_ss"] for r in results]).astype(np.float64)
    tgt = np.concatenate([r["o_tgt"] for r in results]).astype(np.float64)
    rs = np.concatenate([r["o_rs"] for r in results]).astype(np.float64)

    t = float(t)
    s = 1.0 / (t * np.sqrt(ss))
    # sum_c exp(v) = N + (sum_c z)*s + (1/2)*sum v^2, with sum v^2 == 1/t^2
    # exactly; higher Taylor terms are O(1e-9) relative (|v| <= ~0.03).
    lse = np.log(N_CLS + rs * s + 0.5 / (t * t))
    loss = np.float32(np.mean(lse - tgt * s))

    tau = 1e-4 * np.sqrt(ss / N_CLS)
    acc = np.int32(np.sum(tgt >= maxz - tau))
    return loss, acc


def kernel(img_features, txt_features, target_ind, W1, b1, W2, b2,
           logit_scale, t, **_unused):
    img_features = np.asarray(img_features, dtype=np.float32)
    txt_features = np.asarray(txt_features, dtype=np.float32)
    target_ind = np.asarray(target_ind)
    W1 = np.asarray(W1, dtype=np.float32)
    b1 = np.asarray(b1, dtype=np.float32)
    W2 = np.asarray(W2, dtype=np.float32)
    b2 = np.asarray(b2, dtype=np.float32)
    t_val = np.asarray(t).item()
    # logit_scale cancels exactly under the reference's row normalizations.

    nc = get_nc()
    in_maps = make_in_maps(img_features, txt_features, target_ind, W1, b1, W2, b2)
    res = run_bass_kernel_spmd(nc, in_maps, list(range(N_CORES)))
    return postprocess(res.results, target_ind.shape[0], t_val)
